# revision 1
# baseline (speedup 1.0000x reference)
"""Causal self-attention with RoPE on 8 Trainium2 NeuronCores.

Sharding: core c = 4*b + g handles batch b (of 2) and head group g (4 of 16
heads). Each core computes q/k/v projections for its heads, head-local causal
softmax attention, and a partial output projection (Wp columns of its heads);
the host sums the 4 partials per batch.

Layout strategy (per core):
  xT    [C, T]  : x[b] transposed (host) — contraction operand for QKV.
  qT/kT [128,T] : per j-tile (2 heads each), partition = head dim.
  v''   [128,260] x16 : natural layout per t-tile; 65 cols/head =
                  [ones | v_head] so the AV matmul's row 0 accumulates the
                  softmax denominator for free.
  S^T   [s, t]  : scores transposed; exp(0.125*(S+mask)) on ACT -> P^T.
  AV    [65, t] : yT_unnorm (rows 1..64) + r (row 0) per head.
  norm  : 1/r broadcast to [128, t] via a K=4 indicator matmul; 1 DVE mult.
  proj  : outT[e, t] partial = WpT_g.T @ yT  (+ bp on group-leader core).

All matmul operands are float32r (TF32-like rounding, ~12 mantissa bits,
4x faster than fp32 on the PE). Causality is exploited at 128-tile
granularity; diagonal tiles are masked additively before the exp.
"""

import sys

for _p in ("/opt/trn_rl_repo",):
    if _p not in sys.path:
        sys.path.append(_p)

import numpy as np
from contextlib import ExitStack

import concourse.bacc as bacc
import concourse.tile as tile
from concourse import mybir
from concourse.bass_utils import run_bass_kernel_spmd

F32 = mybir.dt.float32
F32R = mybir.dt.float32r
EXP = mybir.ActivationFunctionType.Exp

B, T, C = 2, 2048, 1024
H, D = 16, 64
HG = 4                 # heads per core
JG = HG * D            # 256 j-columns per core
VW = HG * 65           # v'' width (ones col + 64 dims per head)
NKT = C // 128         # 8 contraction tiles
NTT = T // 128         # 16 t-tiles / s-tiles
NC4 = T // 512         # 4 512-chunks
MASK_VAL = -30000.0
SCALE = 1.0 / np.sqrt(D)

_NC_CACHE = None


def build_bass(debug=False, zero_bias=False):
    nc = bacc.Bacc()

    xT = nc.declare_dram_parameter("xT", [C, T], F32, isOutput=False)
    wqT = nc.declare_dram_parameter("wqT", [C, JG], F32, isOutput=False)
    wkT = nc.declare_dram_parameter("wkT", [C, JG], F32, isOutput=False)
    wvT = nc.declare_dram_parameter("wvT", [C, JG], F32, isOutput=False)
    wpT = nc.declare_dram_parameter("wpT", [JG, C], F32, isOutput=False)
    bq_r = nc.declare_dram_parameter("bq_r", [1, JG], F32, isOutput=False)
    bk_r = nc.declare_dram_parameter("bk_r", [1, JG], F32, isOutput=False)
    bv_r = nc.declare_dram_parameter("bv_r", [1, JG], F32, isOutput=False)
    bp_r = nc.declare_dram_parameter("bp_r", [1, C], F32, isOutput=False)
    cosT = nc.declare_dram_parameter("cosT", [128, T], F32, isOutput=False)
    ssT = nc.declare_dram_parameter("ssT", [128, T], F32, isOutput=False)
    bmask = nc.declare_dram_parameter("bmask", [128, 384], F32, isOutput=False)
    ind = nc.declare_dram_parameter("ind", [2, JG], F32, isOutput=False)
    ones_r = nc.declare_dram_parameter("ones_r", [1, 512], F32, isOutput=False)
    vones = nc.declare_dram_parameter("vones", [128, HG], F32, isOutput=False)

    outT = nc.declare_dram_parameter("outT", [C, T], F32, isOutput=True)
    if debug:
        d_qT = [nc.declare_dram_parameter(f"d_qT{j}", [128, T], F32, isOutput=True) for j in range(2)]
        d_kT = [nc.declare_dram_parameter(f"d_kT{j}", [128, T], F32, isOutput=True) for j in range(2)]
        d_yT = [nc.declare_dram_parameter(f"d_yT{j}", [128, T], F32, isOutput=True) for j in range(2)]
        d_v = [nc.declare_dram_parameter(f"d_v{s}", [128, VW], F32, isOutput=True) for s in range(NTT)]
        d_rr4 = nc.declare_dram_parameter("d_rr4", [HG, T], F32, isOutput=True)

    with (
        tile.TileContext(nc) as tc,
        ExitStack() as ctx,
        nc.allow_low_precision(reason="f32r matmul pipeline"),
    ):
        consts = ctx.enter_context(tc.tile_pool(name="consts", bufs=1))

        def load_const(name, dram, shape, dtype=F32R):
            t = consts.tile(shape, dtype, tag=name, name=name)
            src = dram[:] if dtype is F32 else dram[:].bitcast(F32R)
            nc.gpsimd.dma_start(t[:], src)
            return t

        # weights as [128, nk*width]: contraction tile i lives at cols [i*w,(i+1)*w)
        def load_w(name, dram, width, eng=None):
            t = consts.tile([128, NKT * width], F32R, tag=name, name=name)
            (eng or nc.gpsimd).dma_start(
                t[:].rearrange("p (i j) -> p i j", i=NKT),
                dram[:].bitcast(F32R).rearrange("(i p) j -> p i j", p=128),
            )
            return t

        wq_sb = consts.tile([128, NKT * JG], F32R, tag="wq", name="wq")
        # big consts are DMA'd on the sync queue interleaved with the xt
        # stream (see qk loop); small/late consts go via gpsimd SWDGE.
        cos_sb = consts.tile([128, T], F32, tag="cos", name="cos")
        ss_sb = consts.tile([128, T], F32, tag="ss", name="ss")
        wk_sb = consts.tile([128, NKT * JG], F32R, tag="wk", name="wk")
        wv_sb = consts.tile([128, NKT * JG], F32R, tag="wv", name="wv")
        bq_sb = load_const("bq", bq_r, [1, JG])
        bk_sb = load_const("bk", bk_r, [1, JG])
        bv_sb = load_const("bv", bv_r, [1, JG])
        ones_sb = load_const("ones", ones_r, [1, 512])
        bm_sb = load_const("bmask", bmask, [128, 384])
        ind_sb = load_const("ind", ind, [2, JG])
        vones_sb = load_const("vones", vones, [128, HG])

        def _load_w_into(t, dram, width):
            nc.sync.dma_start(
                t[:].rearrange("p (i j) -> p i j", i=NKT),
                dram[:].bitcast(F32R).rearrange("(i p) j -> p i j", p=128),
            )

        def _load_w_ctile(t, dram, width, i):
            nc.sync.dma_start(
                t[:, i * width : (i + 1) * width],
                dram[128 * i : 128 * (i + 1), :].bitcast(F32R),
            )

        # (quarter, i) -> list of extra sync-queue loads to emit at that step
        deferred_loads = {
            (0, 5): [lambda: nc.sync.dma_start(cos_sb[:], cosT[:])],
            (0, 6): [lambda: nc.sync.dma_start(ss_sb[:], ssT[:])],
            (1, 0): [lambda: _load_w_into(wv_sb, wvT, JG)],
        }
        for _i in range(NKT):
            deferred_loads.setdefault((0, _i), []).insert(
                0, (lambda i=_i: (_load_w_ctile(wq_sb, wqT, JG, i),
                                  _load_w_ctile(wk_sb, wkT, JG, i)))
            )

        wp_sb = [None, None]
        for jt in range(2):
            wp_sb[jt] = consts.tile([128, C], F32R, tag=f"wp{jt}", name=f"wp{jt}")
            nc.gpsimd.dma_start(
                wp_sb[jt][:], wpT[128 * jt : 128 * (jt + 1), :].bitcast(F32R)
            )
        bp_sb = load_const("bp", bp_r, [1, C])

        qkv_sb = ctx.enter_context(tc.tile_pool(name="qkv", bufs=1))
        qT_sb = [qkv_sb.tile([128, T], F32R, tag=f"qT{j}", name=f"qT{j}") for j in range(2)]
        kT_sb = [qkv_sb.tile([128, T], F32R, tag=f"kT{j}", name=f"kT{j}") for j in range(2)]
        yT_sb = [qkv_sb.tile([128, T], F32R, tag=f"yT{j}", name=f"yT{j}") for j in range(2)]
        v_sb = [qkv_sb.tile([128, VW], F32R, tag=f"v{s}", name=f"v{s}") for s in range(NTT)]
        rr2_sb = [qkv_sb.tile([2, T], F32R, tag=f"rr2{j}", name=f"rr2{j}") for j in range(2)]
        rscr_sb = qkv_sb.tile([1, T], F32, tag="rscr")

        # ---- q/k phase: T-quarters, 4 slots (q-j0,k-j0,q-j1,k-j1) ------
        # Ropes pipeline under the next quarter's matmuls (psum 2 quarters
        # deep). Rotation = 4 partition-swapped ACT copies (pcr) + 3 wide
        # DVE ops.
        xstream = ctx.enter_context(tc.tile_pool(name="xstream", bufs=4))
        rope_pool = ctx.enter_context(tc.tile_pool(name="rope", bufs=3))
        with tc.tile_pool(name="pqk", bufs=8, space="PSUM") as pqk:
            for qtr in range(4):
                tlo = 512 * qtr
                ps4 = [
                    pqk.tile([128, 512], F32, tag="pqk", name="pqk")
                    for _ in range(4)
                ]
                for i in range(NKT):
                    for fn in deferred_loads.pop((qtr, i), []):
                        fn()
                    xt = xstream.tile([128, 512], F32R, tag="xq", name="xq")
                    nc.sync.dma_start(
                        xt[:],
                        xT[128 * i : 128 * (i + 1), tlo : tlo + 512].bitcast(F32R),
                    )
                    for sl, (jt, w_sb) in enumerate(
                        ((0, wq_sb), (0, wk_sb), (1, wq_sb), (1, wk_sb))
                    ):
                        nc.tensor.matmul(
                            ps4[sl][:],
                            w_sb[:, i * JG + 128 * jt : i * JG + 128 * (jt + 1)],
                            xt[:],
                            start=(i == 0),
                            stop=(zero_bias and i == NKT - 1),
                        )
                for sl, (jt, b_sb, dst) in enumerate(
                    (
                        (0, bq_sb, qT_sb),
                        (0, bk_sb, kT_sb),
                        (1, bq_sb, qT_sb),
                        (1, bk_sb, kT_sb),
                    )
                ):
                    p = ps4[sl]
                    if not zero_bias:
                        nc.tensor.matmul(
                            p[:],
                            b_sb[:, 128 * jt : 128 * (jt + 1)],
                            ones_sb[:, :],
                            start=False,
                            stop=True,
                        )
                    # RoPE: dst = p*cos + rotate_half(p)*ss
                    out = dst[jt][:, tlo : tlo + 512]
                    pcr = rope_pool.tile([128, 512], F32, tag="pcr", name="pcr")
                    for h0 in (0, 64):
                        a_, b_, c_ = h0, h0 + 32, h0 + 64
                        nc.scalar.copy(pcr[a_:b_, :], p[b_:c_, :])
                        nc.scalar.copy(pcr[b_:c_, :], p[a_:b_, :])
                    nc.vector.tensor_mul(out, p[:], cos_sb[:, tlo : tlo + 512])
                    rot = rope_pool.tile([128, 512], F32R, tag="rot", name="rot")
                    nc.vector.tensor_mul(rot[:], pcr[:], ss_sb[:, tlo : tlo + 512])
                    nc.vector.tensor_add(out, out, rot[:])

        # ---- v phase: natural layout, one accumulation group per bank --
        with tc.tile_pool(name="pv", bufs=4, space="PSUM") as pv:
            for tt in range(NTT):
                # column slab of xT for this t-tile: [128, 8 x 128]
                xc = xstream.tile([128, C], F32R, tag="xs", name="xs")
                nc.sync.dma_start(
                    xc[:].rearrange("p (i t) -> p i t", i=NKT),
                    xT[:, 128 * tt : 128 * (tt + 1)]
                    .bitcast(F32R)
                    .rearrange("(i p) t -> p i t", p=128),
                )

                p = pv.tile([128, JG], F32, tag="pv", name="pv")
                for i in range(NKT):
                    nc.tensor.matmul(
                        p[:],
                        xc[:, 128 * i : 128 * (i + 1)],
                        wv_sb[:, i * JG : (i + 1) * JG],
                        start=(i == 0),
                        stop=(zero_bias and i == NKT - 1),
                    )
                if not zero_bias:
                    nc.tensor.matmul(
                        p[:], ones_sb[:, :128], bv_sb[:, :], start=False, stop=True
                    )
                vv = v_sb[tt][:].rearrange("p (h w) -> p h w", h=HG)
                nc.gpsimd.tensor_copy(
                    vv[:, :, 64:65], vones_sb[:].rearrange("p (h w) -> p h w", w=1)
                )
                nc.vector.tensor_copy(
                    vv[:, :, 0:64], p[:].rearrange("p (h w) -> p h w", h=HG)
                )

        # ---- attention: two T-half passes (pass A needs only half-0 rope)
        with (
            tc.tile_pool(name="pst", bufs=3, space="PSUM") as pst,
            tc.tile_pool(name="pav", bufs=1, space="PSUM") as pav,
            tc.tile_pool(name="ppt", bufs=4) as ppt,
        ):
            for lo in (0, 1024):
                nk = (lo + 1024) // 128  # s-tiles in this pass
                for hl in range(HG):
                    jt, m = hl // 2, hl % 2
                    qh = qT_sb[jt][64 * m : 64 * (m + 1), :]
                    kh = kT_sb[jt][64 * m : 64 * (m + 1), :]
                    p_av = pav.tile([65, 1024], F32, tag="pav", name="pav")
                    ppts = [None] * nk

                    def emit_st(k, ppts=ppts, qh=qh, kh=kh):
                        t0 = 128 * k
                        a = max(t0 - lo, 0)
                        estart = min(a, 256) if a < 512 else 512 + min(a - 512, 256)
                        p_st = pst.tile([128, 1024], F32, tag="pst", name="pst")
                        for s5 in range(2):
                            slo = lo + 512 * s5
                            if slo + 512 <= t0:
                                continue
                            sa = min(max(t0 - slo, 0), 256)
                            nc.tensor.matmul(
                                p_st[:, 512 * s5 + sa : 512 * (s5 + 1)],
                                kh[:, 128 * k : 128 * (k + 1)],
                                qh[:, slo + sa : slo + 512],
                                start=True,
                                stop=True,
                            )
                        pt = ppt.tile([128, 1024], F32R, tag="ppt", name="ppt")
                        nc.scalar.activation(
                            pt[:, estart:], p_st[:, estart:], EXP, scale=float(SCALE)
                        )
                        if t0 >= lo:
                            w = a + 128 - estart
                            nc.vector.tensor_mul(
                                pt[:, estart : a + 128],
                                pt[:, estart : a + 128],
                                bm_sb[:, 384 - w : 384],
                            )
                        ppts[k] = pt

                    def emit_av(k, ppts=ppts, p_av=p_av, hl=hl, nk=nk):
                        t0 = 128 * k
                        vh = v_sb[k][:, 65 * hl : 65 * (hl + 1)]
                        for cs in (lo // 512, lo // 512 + 1):
                            slo = 512 * cs
                            if slo + 512 <= t0:
                                continue
                            sa = min(max(t0 - slo, 0), 256)
                            off = slo - lo + sa
                            nc.tensor.matmul(
                                p_av[:, off : slo - lo + 512],
                                vh,
                                ppts[k][:, off : off + 512 - sa],
                                start=(k == 0),
                                stop=(k == min(4 * cs + 3, nk - 1)),
                            )

                    for k in range(nk):
                        emit_st(k)
                        if k > 0:
                            emit_av(k - 1)
                    emit_av(nk - 1)

                    nc.vector.tensor_copy(
                        yT_sb[jt][64 * m : 64 * (m + 1), lo : lo + 1024],
                        p_av[0:64, :],
                    )
                    nc.vector.tensor_copy(
                        rscr_sb[0:1, lo : lo + 1024].bitcast(F32R), p_av[64:65, :]
                    )
                    nc.sync.dma_start(
                        rr2_sb[jt][m : m + 1, lo : lo + 1024],
                        rscr_sb[0:1, lo : lo + 1024].bitcast(F32R),
                    )


        # ---- normalize (yT /= r per head) ------------------------------
        with (
            tc.tile_pool(name="prb", bufs=2, space="PSUM") as prb,
            tc.tile_pool(name="rb", bufs=2) as rb_pool,
        ):
            for jt in range(2):
                for c4 in range(NC4):
                    pn = prb.tile([128, 512], F32, tag="prb", name="prb")
                    nc.tensor.matmul(
                        pn[:],
                        ind_sb[:, 128 * jt : 128 * (jt + 1)],
                        rr2_sb[jt][:, 512 * c4 : 512 * (c4 + 1)],
                        start=True,
                        stop=True,
                    )
                    rb = rb_pool.tile([128, 512], F32, tag="rb", name="rb")
                    nc.vector.reciprocal_approx_fast(out=rb[:], in_=pn[:])
                    sl = yT_sb[jt][:, 512 * c4 : 512 * (c4 + 1)]
                    nc.vector.tensor_mul(sl, sl, rb[:].bitcast(F32R))

        if debug:
            for j in range(2):
                nc.sync.dma_start(d_qT[j][:], qT_sb[j][:].bitcast(F32))
                nc.sync.dma_start(d_kT[j][:], kT_sb[j][:].bitcast(F32))
                nc.sync.dma_start(d_yT[j][:], yT_sb[j][:].bitcast(F32))
            for s in range(NTT):
                nc.sync.dma_start(d_v[s][:], v_sb[s][:].bitcast(F32))
            nc.sync.dma_start(d_rr4[:], rr4_sb[:].bitcast(F32))

        # ---- output projection (transposed, partial) -------------------
        with (
            tc.tile_pool(name="pp", bufs=4, space="PSUM") as pp,
            tc.tile_pool(name="ostage", bufs=2) as ostage,
        ):
            for et in range(8):
                o = ostage.tile([128, T], F32, tag="ostage")
                for c4 in range(NC4):
                    p = pp.tile([128, 512], F32, tag="pp")
                    for jt in range(2):
                        nc.tensor.matmul(
                            p[:],
                            wp_sb[jt][:, 128 * et : 128 * (et + 1)],
                            yT_sb[jt][:, 512 * c4 : 512 * (c4 + 1)],
                            start=(jt == 0),
                            stop=False,
                        )
                    nc.tensor.matmul(
                        p[:],
                        bp_sb[:, 128 * et : 128 * (et + 1)],
                        ones_sb[:, :],
                        start=False,
                        stop=True,
                    )
                    if c4 % 2 == 0:
                        nc.scalar.copy(o[:, 512 * c4 : 512 * (c4 + 1)], p[:])
                    else:
                        nc.vector.tensor_copy(o[:, 512 * c4 : 512 * (c4 + 1)], p[:])
                nc.sync.dma_start(outT[128 * et : 128 * (et + 1), :], o[:])

    nc.finalize()
    return nc


def _rope_tables():
    inv_freq = 1.0 / (10000.0 ** (np.arange(0, D, 2, dtype=np.float32) / D))
    t = np.arange(T, dtype=np.float32)
    freqs = t[:, None] * inv_freq[None, :]              # [T, 32]
    emb = np.concatenate([freqs, freqs], axis=1)        # [T, 64]
    cos = np.cos(emb).astype(np.float32).T              # [64, T]
    sin = np.sin(emb).astype(np.float32).T              # [64, T]
    # rotate_half signs at destination rows: rot[d<32] = -q[d+32]*sin[d]
    ss = np.concatenate([-sin[:32], sin[32:]], axis=0)
    cosT = np.concatenate([cos, cos], axis=0)           # [128, T] (2 heads)
    ssT = np.concatenate([ss, ss], axis=0)              # [128, T]
    return np.ascontiguousarray(cosT), np.ascontiguousarray(ssT)


def _host_inputs(x, Wq, bq, Wk, bk, Wv, bv, Wp, bp):
    cosT, ssT = _rope_tables()
    s = np.arange(128)[:, None]
    u = np.arange(384)[None, :]
    bmask = ((u - 256) >= s).astype(np.float32)
    ind = np.zeros((2, JG), np.float32)
    for j in range(JG):
        ind[(j // 64) % 2, j] = 1.0
    ones_r = np.ones((1, 512), np.float32)

    maps = []
    for b in range(B):
        for g in range(4):
            J = slice(g * JG, (g + 1) * JG)
            maps.append(
                {
                    "xT": np.ascontiguousarray(x[b].T),
                    "wqT": np.ascontiguousarray(Wq[J, :].T),
                    "wkT": np.ascontiguousarray(Wk[J, :].T),
                    "wvT": np.ascontiguousarray(Wv[J, :].T),
                    "wpT": np.ascontiguousarray(Wp[:, J].T),
                    "bq_r": bq[None, J].astype(np.float32),
                    "bk_r": bk[None, J].astype(np.float32),
                    "bv_r": bv[None, J].astype(np.float32),
                    "bp_r": (bp if g == 0 else np.zeros_like(bp))[None, :].astype(
                        np.float32
                    ),
                    "cosT": cosT,
                    "ssT": ssT,
                    "bmask": bmask,
                    "ind": ind,
                    "ones_r": ones_r,
                    "vones": np.ones((128, HG), np.float32),
                }
            )
    return maps


def kernel(x, Wq, bq, Wk, bk, Wv, bv, Wp, bp, _trace=False):
    global _NC_CACHE
    x, Wq, bq, Wk, bk, Wv, bv, Wp, bp = (
        np.asarray(a, np.float32) for a in (x, Wq, bq, Wk, bk, Wv, bv, Wp, bp)
    )
    zb = not (np.any(bq) or np.any(bk) or np.any(bv))
    if _NC_CACHE is None or _NC_CACHE[1] != zb:
        _NC_CACHE = (build_bass(zero_bias=zb), zb)
    maps = _host_inputs(x, Wq, bq, Wk, bk, Wv, bv, Wp, bp)
    res = run_bass_kernel_spmd(_NC_CACHE[0], maps, list(range(8)), trace=_trace)
    out = np.empty((B, T, C), np.float32)
    for b in range(B):
        acc = res.results[4 * b]["outT"].copy()
        for g in range(1, 4):
            acc += res.results[4 * b + g]["outT"]
        out[b] = acc.T
    if _trace:
        return out, res
    return out



# revision 34
# speedup vs baseline: 2.3224x; 2.3224x over previous
"""Causal self-attention with RoPE on 8 Trainium2 NeuronCores.

Sharding: core c = 4*b + g handles batch b (of 2) and head group g (4 of 16
heads). Each core computes q/k/v projections for its heads, head-local causal
softmax attention, and a partial output projection (Wp columns of its heads);
the host sums the 4 partials per batch.

v4 layout (all matmul operands bf16):
  xT    [C, T]  : x[b] transposed (host), bf16 — loaded ONCE, fully SBUF
                  resident; feeds both the QKV projections and the V tiles.
  qT/kT [128,T] : per j-tile (2 heads each), partition = head dim, bf16.
  RoPE  : ACT does the PSUM->bf16 cast copy + 1 offset block copy; DVE does
          3 offset block copies (4x) + mul cos + mul sin + add (2x).
  v''   [128,65*4] : natural layout per t-tile; col 64 of each head slot is
          ones so the AV matmul's row 64 accumulates the softmax denominator.
  S^T   [s, t]  : scores transposed in PSUM; exp(0.125*S) on ACT -> P^T bf16;
          causality exact to 128 (bf16 matmul has no small-N penalty).
  mask  : diagonal-tile causal mask multiplied on gpsimd (Pool) in bf16.
  AV    [65, 512] x2 chunks : yT_unnorm (rows 0..64) + r (row 64) per head;
          chunk 0 drains early (DVE copies to yT / rsc).
  norm  : 1/r broadcast to [128, t] via ones-lhsT matmuls + fast reciprocal.
  proj  : outT[e,t] partial per (et, 512-chunk), bf16 ostage, 32 outT DMAs.

Scheduling: one persistent PSUM pool (tags st 2x[128,1024], av 2x[128,512],
ut/qq 1x[128,512] each = 8 banks).  Work is emitted as ~0.4-0.9us "units";
QKV quarters 2,3 + V tiles 8-15 are interleaved as fill units into the
ACT-bound attention pass-A heads, and norm/proj units into pass B, keeping
PE (the busiest engine) running nearly gap-free.
"""

import sys

for _p in ("/opt/trn_rl_repo",):
    if _p not in sys.path:
        sys.path.append(_p)

import numpy as np
import ml_dtypes
from contextlib import ExitStack

import concourse.bacc as bacc
import concourse.tile as tile
from concourse import mybir
from concourse.bass_utils import run_bass_kernel_spmd

F32 = mybir.dt.float32
F32R = mybir.dt.float32r
BF16 = mybir.dt.bfloat16
EXP = mybir.ActivationFunctionType.Exp
BF = ml_dtypes.bfloat16

B, T, C = 2, 2048, 1024
H, D = 16, 64
HG = 4                 # heads per core
JG = HG * D            # 256 j-columns per core
VW = HG * 65           # v'' width (64 dims + ones col per head)
NKT = C // 128         # 8 contraction tiles
NTT = T // 128         # 16 t-tiles / s-tiles
SCALE = 1.0 / np.sqrt(D)

_NC_CACHE = None


def build_bass(zero_bias=False):
    nc = bacc.Bacc()

    xT = nc.declare_dram_parameter("xT", [C, T], BF16, isOutput=False)
    wqT = nc.declare_dram_parameter("wqT", [C, JG], BF16, isOutput=False)
    wkT = nc.declare_dram_parameter("wkT", [C, JG], BF16, isOutput=False)
    wvT = nc.declare_dram_parameter("wvT", [C, JG], BF16, isOutput=False)
    wpT = nc.declare_dram_parameter("wpT", [JG, C], BF16, isOutput=False)
    bq_r = nc.declare_dram_parameter("bq_r", [1, JG], BF16, isOutput=False)
    bk_r = nc.declare_dram_parameter("bk_r", [1, JG], BF16, isOutput=False)
    bv_r = nc.declare_dram_parameter("bv_r", [1, JG], BF16, isOutput=False)
    bp_r = nc.declare_dram_parameter("bp_r", [1, C], BF16, isOutput=False)
    cosT = nc.declare_dram_parameter("cosT", [128, T], BF16, isOutput=False)
    ssT = nc.declare_dram_parameter("ssT", [128, T], BF16, isOutput=False)
    bmask = nc.declare_dram_parameter("bmask", [128, 384], BF16, isOutput=False)
    ones_r = nc.declare_dram_parameter("ones_r", [1, 512], BF16, isOutput=False)
    vones = nc.declare_dram_parameter("vones", [128, HG], BF16, isOutput=False)

    outT = nc.declare_dram_parameter("outT", [C, T], BF16, isOutput=True)

    with (
        tile.TileContext(nc) as tc,
        ExitStack() as ctx,
        nc.allow_low_precision(reason="bf16 matmul pipeline"),
    ):
        consts = ctx.enter_context(tc.tile_pool(name="consts", bufs=1))

        xs = [consts.tile([128, T], BF16, tag=f"xs{i}", name=f"xs{i}") for i in range(NKT)]
        wq_sb = consts.tile([128, NKT * JG], BF16, tag="wq", name="wq")
        wk_sb = consts.tile([128, NKT * JG], BF16, tag="wk", name="wk")
        wv_sb = consts.tile([128, NKT * JG], BF16, tag="wv", name="wv")
        cos_sb = consts.tile([128, T], BF16, tag="cos", name="cos")
        ss_sb = consts.tile([128, T], BF16, tag="ss", name="ss")
        wp_sb = [consts.tile([128, C], BF16, tag=f"wp{j}", name=f"wp{j}") for j in range(2)]
        bq_sb = consts.tile([1, JG], BF16, tag="bq", name="bq")
        bk_sb = consts.tile([1, JG], BF16, tag="bk", name="bk")
        bv_sb = consts.tile([1, JG], BF16, tag="bv", name="bv")
        bp_sb = consts.tile([1, C], BF16, tag="bp", name="bp")
        ones_sb = consts.tile([1, 512], BF16, tag="ones", name="ones")
        bm_sb = consts.tile([128, 384], BF16, tag="bmask", name="bmask")
        vones_sb = consts.tile([128, HG], BF16, tag="vones", name="vones")

        # ---- input DMA schedule (sync queue drains in emission order) ----
        # first contraction tile of wq/wk on the parallel SWDGE queue so the
        # first matmul can start ~2.5us earlier; sync queue carries the rest.
        nc.gpsimd.dma_start(wq_sb[:, 0:JG], wqT[0:128, :])
        nc.gpsimd.dma_start(wk_sb[:, 0:JG], wkT[0:128, :])
        nc.gpsimd.dma_start(wq_sb[:, JG : 2 * JG], wqT[128:256, :])
        nc.gpsimd.dma_start(wk_sb[:, JG : 2 * JG], wkT[128:256, :])

        def load_w_rest(t, dram):
            nc.sync.dma_start(
                t[:, 2 * JG :].rearrange("p (i j) -> p i j", i=NKT - 2),
                dram[256:, :].rearrange("(i p) j -> p i j", p=128),
            )

        nc.sync.dma_start(xs[0][:, 0:1024], xT[0:128, 0:1024])
        nc.sync.dma_start(xs[1][:, 0:1024], xT[128:256, 0:1024])
        load_w_rest(wq_sb, wqT)
        load_w_rest(wk_sb, wkT)
        for i in range(2, 3):
            nc.sync.dma_start(xs[i][:, 0:1024], xT[128 * i : 128 * (i + 1), 0:1024])
        nc.sync.dma_start(cos_sb[:, 0:512], cosT[:, 0:512])
        nc.sync.dma_start(ss_sb[:, 0:512], ssT[:, 0:512])
        for i in range(3, 6):
            nc.sync.dma_start(xs[i][:, 0:1024], xT[128 * i : 128 * (i + 1), 0:1024])
        nc.sync.dma_start(cos_sb[:, 512:1024], cosT[:, 512:1024])
        nc.sync.dma_start(ss_sb[:, 512:1024], ssT[:, 512:1024])
        for i in range(6, NKT):
            nc.sync.dma_start(xs[i][:, 0:1024], xT[128 * i : 128 * (i + 1), 0:1024])
        nc.sync.dma_start(cos_sb[:, 1024:2048], cosT[:, 1024:2048])
        nc.sync.dma_start(ss_sb[:, 1024:2048], ssT[:, 1024:2048])
        def load_w(t, dram):
            nc.sync.dma_start(
                t[:].rearrange("p (i j) -> p i j", i=NKT),
                dram[:].rearrange("(i p) j -> p i j", p=128),
            )

        load_w(wv_sb, wvT)
        for i in range(NKT):
            nc.sync.dma_start(
                xs[i][:, 1024:2048], xT[128 * i : 128 * (i + 1), 1024:2048]
            )
        for j in range(2):
            nc.sync.dma_start(wp_sb[j][:], wpT[128 * j : 128 * (j + 1), :])

        # small consts on the gpsimd SWDGE queue
        nc.gpsimd.dma_start(bq_sb[:], bq_r[:])
        nc.gpsimd.dma_start(bk_sb[:], bk_r[:])
        nc.gpsimd.dma_start(bv_sb[:], bv_r[:])
        nc.gpsimd.dma_start(bp_sb[:], bp_r[:])
        nc.gpsimd.dma_start(ones_sb[:], ones_r[:])
        nc.gpsimd.dma_start(bm_sb[:], bmask[:])
        nc.gpsimd.dma_start(vones_sb[:], vones[:])

        qkv_sb = ctx.enter_context(tc.tile_pool(name="qkv", bufs=1))
        qT_sb = [qkv_sb.tile([128, T], BF16, tag=f"qT{j}", name=f"qT{j}") for j in range(2)]
        kT_sb = [qkv_sb.tile([128, T], BF16, tag=f"kT{j}", name=f"kT{j}") for j in range(2)]
        yT_sb = [qkv_sb.tile([128, T], BF16, tag=f"yT{j}", name=f"yT{j}") for j in range(2)]
        v_sb = [qkv_sb.tile([128, VW], BF16, tag=f"v{s}", name=f"v{s}") for s in range(NTT)]
        rsc_sb = [qkv_sb.tile([1, T], BF16, tag=f"rsc{h}", name=f"rsc{h}") for h in range(HG)]

        ppool = ctx.enter_context(tc.tile_pool(name="ppool", bufs=1, space="PSUM"))
        rope_pool = ctx.enter_context(tc.tile_pool(name="rope", bufs=3))
        ppt = ctx.enter_context(tc.tile_pool(name="ppt", bufs=4))
        rb_pool = ctx.enter_context(tc.tile_pool(name="rb", bufs=2))
        ostage = ctx.enter_context(tc.tile_pool(name="ostage", bufs=4))

        def st_tile():
            return ppool.tile([128, 1024], F32, tag="st", name="st", bufs=2)

        def av_tile():
            return ppool.tile([128, 512], F32, tag="av", name="av", bufs=2)

        _gp = [0]

        def gp_tile():
            # alternating general-purpose [128,512] PSUM slot (2 total);
            # used only by serialized streams (quarters / V / norm / proj)
            _gp[0] ^= 1
            tag = "qq" if _gp[0] else "ut"
            return ppool.tile([128, 512], F32, tag=tag, name=tag, bufs=1)

        # ---- QKV quarter units -----------------------------------------
        # quarter = 512 t-cols; processed as two jt-blocks of (q,k) slots,
        # each: 8 contraction steps (2 matmuls) + 1 rope unit.
        def rope_slot(p, dst_t, tlo, on_act=True):
            eng = nc.scalar if on_act else nc.vector
            cp = eng.copy if on_act else eng.tensor_copy
            out = dst_t[:, tlo : tlo + 512]
            qc = rope_pool.tile([128, 512], BF16, tag="qc", name="qc")
            cp(qc[:], p)
            qcr = rope_pool.tile([128, 512], BF16, tag="qcr", name="qcr")
            nc.gpsimd.tensor_copy(qcr[0:32, :], qc[32:64, :])
            nc.gpsimd.tensor_copy(qcr[32:64, :], qc[0:32, :])
            nc.vector.tensor_copy(qcr[64:96, :], qc[96:128, :])
            nc.vector.tensor_copy(qcr[96:128, :], qc[64:96, :])
            nc.vector.tensor_mul(out, qc[:], cos_sb[:, tlo : tlo + 512])
            rot = rope_pool.tile([128, 512], BF16, tag="rot", name="rot")
            nc.vector.tensor_mul(rot[:], qcr[:], ss_sb[:, tlo : tlo + 512])
            nc.vector.tensor_add(out, out, rot[:])

        def quarter_units(qtr, direct=False):
            tlo = 512 * qtr
            units = []
            for jt in range(2):
                hs = {}

                def mk_i(i, jt=jt, hs=hs):
                    def u():
                        if i == 0:
                            if direct and jt == 0:
                                st2 = st_tile()
                                hs["pq"] = st2[:, 0:512]
                                hs["pk"] = st2[:, 512:1024]
                            elif direct:
                                hs["pq"] = av_tile()[:]
                                hs["pk"] = gp_tile()[:]
                            else:
                                hs["pq"] = gp_tile()[:]
                                hs["pk"] = gp_tile()[:]
                        for p, w_sb in ((hs["pq"], wq_sb), (hs["pk"], wk_sb)):
                            nc.tensor.matmul(
                                p,
                                w_sb[:, i * JG + 128 * jt : i * JG + 128 * (jt + 1)],
                                xs[i][:, tlo : tlo + 512],
                                start=(i == 0),
                                stop=(zero_bias and i == NKT - 1),
                            )
                    return u

                def mk_rope(jt=jt, hs=hs):
                    def u():
                        for p, b_sb, dst in (
                            (hs["pq"], bq_sb, qT_sb),
                            (hs["pk"], bk_sb, kT_sb),
                        ):
                            if not zero_bias:
                                nc.tensor.matmul(
                                    p,
                                    b_sb[:, 128 * jt : 128 * (jt + 1)],
                                    ones_sb[:, :],
                                    start=False,
                                    stop=True,
                                )
                            rope_slot(p, dst[jt], tlo)
                    return u

                units += [mk_i(i) for i in range(NKT)]
                units.append(mk_rope())
            return units

        # ---- V tile units ----------------------------------------------
        # early tiles drain on ACT (idle then; DVE carries the rope
        # backlog), late tiles on DVE (ACT is exp-saturated by then)
        def vtile_units(t_lo, t_hi):
            def mk(tt):
                def u():
                    p = gp_tile()[:, 0:JG]
                    for i in range(NKT):
                        nc.tensor.matmul(
                            p,
                            xs[i][:, 128 * tt : 128 * (tt + 1)],
                            wv_sb[:, i * JG : (i + 1) * JG],
                            start=(i == 0),
                            stop=(zero_bias and i == NKT - 1),
                        )
                    if not zero_bias:
                        nc.tensor.matmul(
                            p, ones_sb[:, :128], bv_sb[:, :], start=False, stop=True
                        )
                    vv = v_sb[tt][:].rearrange("p (h w) -> p h w", h=HG)
                    nc.gpsimd.tensor_copy(
                        vv[:, :, 64:65], vones_sb[:].rearrange("p (h w) -> p h w", w=1)
                    )
                    if tt < 8:
                        nc.scalar.copy(
                            vv[:, :, 0:64], p.rearrange("p (h w) -> p h w", h=HG)
                        )
                    else:
                        nc.vector.tensor_copy(
                            vv[:, :, 0:64], p.rearrange("p (h w) -> p h w", h=HG)
                        )
                return u

            return [mk(tt) for tt in range(t_lo, t_hi)]

        # ---- norm / proj units (per 512-chunk) -------------------------
        def norm_unit(c4):
            def u():
                for jt in range(2):
                    pn = gp_tile()
                    for m in range(2):
                        nc.tensor.matmul(
                            pn[64 * m : 64 * (m + 1), :],
                            ones_sb[:, 0:64],
                            rsc_sb[2 * jt + m][:, 512 * c4 : 512 * (c4 + 1)],
                            start=True,
                            stop=True,
                        )
                    rb = rb_pool.tile([128, 512], F32, tag="rb", name="rb")
                    nc.vector.reciprocal_approx_fast(out=rb[:], in_=pn[:])
                    sl = yT_sb[jt][:, 512 * c4 : 512 * (c4 + 1)]
                    nc.vector.tensor_mul(sl, sl, rb[:])
            return u

        def proj_unit(c4, et):
            def u():
                # tail chunks run after the exp stream ends: use the then-free
                # av ring and ACT for the copy; earlier chunks share gp/DVE
                p = av_tile() if c4 >= 2 else gp_tile()
                for jt in range(2):
                    nc.tensor.matmul(
                        p[:],
                        wp_sb[jt][:, 128 * et : 128 * (et + 1)],
                        yT_sb[jt][:, 512 * c4 : 512 * (c4 + 1)],
                        start=(jt == 0),
                        stop=(zero_bias and jt == 1),
                    )
                if not zero_bias:
                    nc.tensor.matmul(
                        p[:],
                        bp_sb[:, 128 * et : 128 * (et + 1)],
                        ones_sb[:, :],
                        start=False,
                        stop=True,
                    )
                o = ostage.tile([128, 512], BF16, tag="ostage", name="ostage")
                if c4 >= 2:
                    nc.scalar.copy(o[:], p[:])
                else:
                    nc.vector.tensor_copy(o[:], p[:])
                nc.sync.dma_start(
                    outT[128 * et : 128 * (et + 1), 512 * c4 : 512 * (c4 + 1)], o[:]
                )
            return u

        # ---- attention head (one T-half pass for one head) -------------
        def attn_head(lo, hl, fills=None, fill_ks=()):
            nk = (lo + 1024) // 128  # s-tiles in this pass
            jt, m = hl // 2, hl % 2
            qh = qT_sb[jt][64 * m : 64 * (m + 1), :]
            kh = kT_sb[jt][64 * m : 64 * (m + 1), :]
            # one AV accumulator per 512-chunk so chunk 0 can drain early
            pavs = [av_tile()[0:65, :], av_tile()[0:65, :]]
            ppts = [None] * nk

            def emit_st(k):
                t0 = 128 * k
                a = max(t0 - lo, 0)
                p_st = st_tile()
                for s5 in range(2):
                    slo = lo + 512 * s5
                    if slo + 512 <= t0:
                        continue
                    sa = min(max(t0 - slo, 0), 384)
                    nc.tensor.matmul(
                        p_st[:, 512 * s5 + sa : 512 * (s5 + 1)],
                        kh[:, 128 * k : 128 * (k + 1)],
                        qh[:, slo + sa : slo + 512],
                        start=True,
                        stop=True,
                    )
                pt = ppt.tile([128, 1024], BF16, tag="ppt", name="ppt")
                nc.scalar.activation(
                    pt[:, a:], p_st[:, a:], EXP, scale=float(SCALE)
                )
                if t0 >= lo:
                    nc.gpsimd.tensor_mul(
                        pt[:, a : a + 128],
                        pt[:, a : a + 128],
                        bm_sb[:, 256:384],
                    )
                ppts[k] = pt

            def drain(cs):
                glo = 512 * cs
                nc.vector.tensor_copy(
                    yT_sb[jt][64 * m : 64 * (m + 1), glo : glo + 512],
                    pavs[cs - lo // 512][0:64, :],
                )
                nc.vector.tensor_copy(
                    rsc_sb[hl][0:1, glo : glo + 512],
                    pavs[cs - lo // 512][64:65, :],
                )

            def emit_av(k):
                t0 = 128 * k
                vh = v_sb[k][:, 65 * hl : 65 * (hl + 1)]
                for cs in (lo // 512, lo // 512 + 1):
                    slo = 512 * cs
                    if slo + 512 <= t0:
                        continue
                    sa = min(max(t0 - slo, 0), 384)
                    off = slo - lo + sa
                    klast = min(4 * cs + 3, nk - 1)
                    nc.tensor.matmul(
                        pavs[cs - lo // 512][:, sa:],
                        vh,
                        ppts[k][:, off : off + 512 - sa],
                        start=(k == 0),
                        stop=(k == klast),
                    )
                    if k == klast:
                        drain(cs)

            for k in range(nk):
                emit_st(k)
                if k > 0:
                    emit_av(k - 1)
                if fills:
                    for _ in range(fill_ks.count(k)):
                        if fills:
                            fills.pop(0)()
            emit_av(nk - 1)

        # ---- emission schedule -----------------------------------------
        for qtr in (0, 1):
            for u in quarter_units(qtr, direct=True):
                u()
        # pass A: heads ACT-bound; fill PE with V tiles 0-11 + quarters 2,3
        fillsA = vtile_units(0, 8) + quarter_units(2) + quarter_units(3) + vtile_units(8, 12)
        ka = (0, 0, 1, 1, 2, 2, 3, 3, 4, 4, 5, 5, 6, 6, 7, 7)
        attn_head(0, 0, fillsA, ka)
        attn_head(0, 1, fillsA, ka)
        attn_head(0, 2, fillsA, ka)
        attn_head(0, 3, fillsA, ka)
        for u in fillsA:
            u()
        fillsA.clear()

        # pass B: fill with half-0 norm + proj; tail work into head 3
        fillsB = (
            [norm_unit(0), norm_unit(1)]
            + [proj_unit(0, et) for et in range(8)]
            + [proj_unit(1, et) for et in range(8)]
        )
        fillsB = vtile_units(12, 16) + fillsB
        attn_head(1024, 0, fillsB, (2, 4, 6, 8, 10))
        attn_head(1024, 1, fillsB, (2, 4, 6, 8, 10, 12, 14))
        attn_head(1024, 2, fillsB, (1, 2, 3, 4, 6, 8, 10, 12, 14))
        for u in fillsB:
            u()
        # tail fills for head 3: only at k >= 13, i.e. after its chunk-2
        # drain (emitted in the k=12 iteration) exists in the DVE queue
        fills3 = [norm_unit(2)] + [proj_unit(2, et) for et in range(5)]
        attn_head(1024, 3, fills3, (13, 13, 14, 14, 15, 15))
        for u in fills3:
            u()
        for et in range(5, 8):
            proj_unit(2, et)()
        norm_unit(3)()
        for et in range(8):
            proj_unit(3, et)()

    nc.finalize()
    return nc


def _rope_tables():
    inv_freq = 1.0 / (10000.0 ** (np.arange(0, D, 2, dtype=np.float64) / D))
    t = np.arange(T, dtype=np.float64)
    freqs = t[:, None] * inv_freq[None, :]              # [T, 32]
    emb = np.concatenate([freqs, freqs], axis=1)        # [T, 64]
    cos = np.cos(emb).astype(np.float32).T              # [64, T]
    sin = np.sin(emb).astype(np.float32).T              # [64, T]
    # rotate_half signs at destination rows: rot[d<32] = -q[d+32]*sin[d]
    ss = np.concatenate([-sin[:32], sin[32:]], axis=0)
    cosT = np.concatenate([cos, cos], axis=0)           # [128, T] (2 heads)
    ssT = np.concatenate([ss, ss], axis=0)              # [128, T]
    return cosT.astype(BF), ssT.astype(BF)


def _host_inputs(x, Wq, bq, Wk, bk, Wv, bv, Wp, bp):
    cosT, ssT = _rope_tables()
    s = np.arange(128)[:, None]
    u = np.arange(384)[None, :]
    bmask = ((u - 256) >= s).astype(BF)
    ones_r = np.ones((1, 512), BF)

    maps = []
    for b in range(B):
        for g in range(4):
            J = slice(g * JG, (g + 1) * JG)
            maps.append(
                {
                    "xT": np.ascontiguousarray(x[b].T).astype(BF),
                    "wqT": np.ascontiguousarray(Wq[J, :].T).astype(BF),
                    "wkT": np.ascontiguousarray(Wk[J, :].T).astype(BF),
                    "wvT": np.ascontiguousarray(Wv[J, :].T).astype(BF),
                    "wpT": np.ascontiguousarray(Wp[:, J].T).astype(BF),
                    "bq_r": bq[None, J].astype(BF),
                    "bk_r": bk[None, J].astype(BF),
                    "bv_r": bv[None, J].astype(BF),
                    "bp_r": (bp if g == 0 else np.zeros_like(bp))[None, :].astype(BF),
                    "cosT": cosT,
                    "ssT": ssT,
                    "bmask": bmask,
                    "ones_r": ones_r,
                    "vones": np.ones((128, HG), BF),
                }
            )
    return maps


def kernel(x, Wq, bq, Wk, bk, Wv, bv, Wp, bp, _trace=False):
    global _NC_CACHE
    x, Wq, bq, Wk, bk, Wv, bv, Wp, bp = (
        np.asarray(a, np.float32) for a in (x, Wq, bq, Wk, bk, Wv, bv, Wp, bp)
    )
    zb = not (np.any(bq) or np.any(bk) or np.any(bv) or np.any(bp))
    if _NC_CACHE is None or _NC_CACHE[1] != zb:
        _NC_CACHE = (build_bass(zero_bias=zb), zb)
    maps = _host_inputs(x, Wq, bq, Wk, bk, Wv, bv, Wp, bp)
    res = run_bass_kernel_spmd(_NC_CACHE[0], maps, list(range(8)), trace=_trace)
    out = np.empty((B, T, C), np.float32)
    for b in range(B):
        acc = res.results[4 * b]["outT"].astype(np.float32)
        for g in range(1, 4):
            acc += res.results[4 * b + g]["outT"].astype(np.float32)
        out[b] = acc.T
    if _trace:
        return out, res
    return out
